# revision 13
# baseline (speedup 1.0000x reference)
"""Trainium2 Bass kernel: multi-table embedding gather (pooling=NONE).

Reference computation (hardcoded shapes):
    indices: [F=4, BL=204800] int   (values in [0, V))
    tables:  [F=4, V=1e6, D=64] f32
    out[f]   = tables[PERM[f]][indices[PERM[f]]]   -> [4, 204800, 64] f32
    PERM = [2, 0, 3, 1]

Strategy (model/table-parallel, per the sharding hint):
  * Fold the table permutation into global row ids g = PERM[f]*V + idx over a
    flat [4M, 64] table.
  * Shard the flat table row-wise across the 8 cores (500,000 rows each).
    The host routes every lookup to the core owning its row, bucketing each
    core's lookups by 32,768-row window so the on-core gather can use the
    high-throughput int16 `dma_gather` SWDGE instruction in single-packet
    mode (~0.1us emission per 1024-idx gather; multi-packet mode measured
    ~3ns/desc on the Pool engine and serializes the whole kernel).
  * Each core gathers its (padded) buckets window-by-window into SBUF,
    casts f32 -> fp16 on the otherwise-idle Activation engine, and streams
    the halved bytes to a contiguous fp16 staging output with large HWDGE
    DMAs (writeback HBM traffic 28MB -> 14MB per core; fp16 roundtrip error
    ~5e-4 rel, far inside the 2e-2 gate).
  * The host applies the recorded inverse permutation to scatter staged rows
    into the final [4, 204800, 64] f32 output (host-side unshard + upcast).
"""

import sys

import numpy as np

for _p in ("/opt/trn_rl_repo",):
    if _p not in sys.path:
        sys.path.insert(0, _p)

F = 4
V = 1_000_000
D = 64
BL = 204_800
PERM = (2, 0, 3, 1)

N_CORES = 8
P = 128
ROWS_TOTAL = F * BL                   # 819,200 lookups
SHARD = F * V // N_CORES              # 500,000 table rows per core
WIN = 32_768                          # int16-addressable window
N_FULL_WIN = SHARD // WIN             # 15 full windows
LAST_WIN_ROWS = SHARD - N_FULL_WIN * WIN  # 8,480
N_WIN = N_FULL_WIN + 1                # 16 windows per core

# Per-window bucket capacity (static padding; lookups are uniform so bucket
# sizes concentrate tightly: full-window mean 6711 sigma 82, last-window mean
# 1737 sigma 42).
PAD_FULL = 7_168                      # 56 * 128
PAD_LAST = 2_048                      # 16 * 128
PADS = [PAD_FULL] * N_FULL_WIN + [PAD_LAST]
COLS = [p // P for p in PADS]         # dst free-dim blocks per window
STAGE_ROWS = sum(PADS)                # 109,568 staged rows per core
IDX_COLS = sum(p // 16 for p in PADS)  # int16 idx columns: 6,848
WIN_ROWS = [WIN] * N_FULL_WIN + [LAST_WIN_ROWS]

NBUF = 6                # window dst tiles in flight (21KB/partition each)
GRANULE = 1024          # idxs per dma_gather (single-packet limit: 64 desc/engine)
GCOLS = GRANULE // P    # 8 dst free-dim blocks per sub-gather
N_SWDGE_QUEUES = 4
DMA_SCRATCH = 65536
NSUB = sum(p // GRANULE for p in PADS)   # 107 sub-gathers per core


def build_nc():
    """Per-core SPMD program: dma_gather windows + fp16 cast + writebacks.

    Raw bass (no TileContext): Tile's SWDGE completion tracking rotates 8
    global DMASW lane sems and makes gather i wait for gather i-8's DMA
    completion before issuing, pacing the kernel at ~2.7us/gather (~290us).
    With manual semaphores the gathers issue back-to-back (~0.1us each),
    all 27 per queue fit the 4096-descriptor ring carveout, and the SDMA
    engines stay saturated. Pipeline (NBUF-deep, per window w):
      Pool:   gathers(w) gated on idx-load(w) and writeback(w-NBUF)
      ACT:    cast(w) f32->fp16 gated on gathers(w) and writeback(w-NBUF)
      SP:     writeback(w) gated on cast(w)
    """
    import concourse.bacc as bacc
    import concourse.mybir as mybir

    nc = bacc.Bacc(
        None,
        num_swdge_queues=N_SWDGE_QUEUES,
        dynamic_dma_scratch_size=DMA_SCRATCH,
    )
    tabs = [
        nc.declare_dram_parameter(
            f"tab{w}", [WIN_ROWS[w], D], mybir.dt.float32, isOutput=False
        )
        for w in range(N_WIN)
    ]
    idx_in = nc.declare_dram_parameter(
        "idx", [P, IDX_COLS], mybir.dt.int16, isOutput=False
    )
    cnt_in = nc.declare_dram_parameter(
        "cnt", [P, NSUB], mybir.dt.int32, isOutput=False
    )
    out = nc.declare_dram_parameter(
        "out", [STAGE_ROWS, D], mybir.dt.float16, isOutput=True
    )

    idx_off = np.cumsum([0] + [p // 16 for p in PADS]).tolist()
    stage_off = np.cumsum([0] + PADS).tolist()

    idx_tile = nc.alloc_sbuf_tensor("idx_sb", [P, IDX_COLS], mybir.dt.int16)
    cnt_tile = nc.alloc_sbuf_tensor("cnt_sb", [P, NSUB], mybir.dt.int32)
    datas = [
        nc.alloc_sbuf_tensor(f"data{i}", [P, COLS[0] * D], mybir.dt.float32)
        for i in range(NBUF)
    ]
    halves = [
        nc.alloc_sbuf_tensor(f"half{i}", [P, COLS[0] * D], mybir.dt.float16)
        for i in range(NBUF)
    ]

    regs = [nc.alloc_register(mybir.EngineType.Pool, f"cnt_reg{i}") for i in range(24)]
    csem = nc.alloc_semaphore("csem")   # cnt load,          +16
    isems = [nc.alloc_semaphore(f"isem{w}") for w in range(N_WIN)]
    wsems = [nc.alloc_semaphore(f"wsem{i}") for i in range(NBUF)]
    asem = nc.alloc_semaphore("asem")   # casts,             +1 each
    psem = nc.alloc_semaphore("psem")   # desc-gen commits,  +1 each
    # one completion sem per window: all its gathers inc +16 and the cast
    # waits for the full sum, so intermediate values are never waited on and
    # out-of-order completion across gathers is harmless.
    gsems = [nc.alloc_semaphore(f"gsem{w}") for w in range(N_WIN)]

    g_of_w = [PADS[w] // GRANULE for w in range(N_WIN)]

    with nc.Block("main") as blk:

        @blk.sync
        def _(sync):
            sync.dma_start(out=cnt_tile[:], in_=cnt_in[:]).then_inc(csem, 16)
            # split the idx load per window so window 0 can start gathering
            # without waiting for the full 1.75MB index transfer
            for w in range(N_WIN):
                sync.dma_start(
                    out=idx_tile[:, idx_off[w] : idx_off[w + 1]],
                    in_=idx_in[:, idx_off[w] : idx_off[w + 1]],
                ).then_inc(isems[w], 16)
            for w in range(N_WIN):
                sync.wait_ge(asem, w + 1)
                win_ap = out[stage_off[w] : stage_off[w + 1], :].rearrange(
                    "(p c) d -> p (c d)", p=P
                )
                sync.dma_start(
                    out=win_ap[:], in_=halves[w % NBUF][:, : COLS[w] * D]
                ).then_inc(wsems[w % NBUF], 16)
            for i in range(NBUF):
                sync.wait_ge(wsems[i], 16 * ((N_WIN - 1 - i) // NBUF + 1))

        @blk.gpsimd
        def _(g):
            # PREPARE_ONLY + trigger: the plain-gather ucode waits for the
            # previous same-queue DMA's completion before posting (~8.6us,
            # max 4 in flight). Preps just write descriptors into the queue
            # ring (28 x 65 desc/queue << 4096-desc carveout) and the
            # trigger doorbell fires them, so all windows' packets pipeline
            # on the SDMA engines back to back.
            g_idx = 0
            g.wait_ge(csem, 16)
            for w in range(N_WIN):
                g.wait_ge(isems[w], 16)
                for s in range(g_of_w[w]):
                    c0 = idx_off[w] + s * (GRANULE // 16)
                    f0 = s * GRANULE // P * D
                    # Runtime count register: the ucode only emits
                    # descriptors for the valid (deduped) prefix; trailing
                    # -1 idx slots are skipped. 1024 idxs = 64 desc/engine =
                    # the single-packet limit.
                    reg = regs[g_idx % len(regs)]
                    # whole window on one queue: an SWDGE completion sem is
                    # locked to a single queue, and gsem[w] is per-window.
                    # NBUF=6 windows in flight still cover all 4 queues.
                    q = w % N_SWDGE_QUEUES
                    g.reg_load(reg, cnt_tile[0:1, g_idx : g_idx + 1])
                    g.dma_gather(
                        datas[w % NBUF][:, f0 : f0 + GCOLS * D].rearrange(
                            "p (c d) -> p c d", d=D
                        ),
                        tabs[w][:],
                        idx_tile[:, c0 : c0 + GRANULE // 16],
                        GRANULE,
                        reg,
                        D,
                        prepare_only=True,
                        sem=gsems[w],
                        single_packet=True,
                        queue_num=q,
                    ).then_inc(psem, 1)
                    g_idx += 1
                # tile-reuse gates FIRING, not desc-gen: wait right before
                # the trigger so later windows' preps still post early.
                if w >= NBUF:
                    g.wait_ge(wsems[w % NBUF], 16 * ((w - NBUF) // NBUF + 1))
                g.wait_ge(psem, g_idx)
                g.trigger_dma(count=None, queue_num=w % N_SWDGE_QUEUES)

        @blk.scalar
        def _(sc):
            for w in range(N_WIN):
                sc.wait_ge(gsems[w], 16 * g_of_w[w])
                if w >= NBUF:
                    sc.wait_ge(wsems[w % NBUF], 16 * ((w - NBUF) // NBUF + 1))
                sc.copy(
                    halves[w % NBUF][:, : COLS[w] * D],
                    datas[w % NBUF][:, : COLS[w] * D],
                ).then_inc(asem, 1)

    nc.compile()
    return nc


def route(indices):
    """Host-side routing: global ids -> per-core window buckets.

    Returns (idx_inputs [N_CORES, P, IDX_COLS] int16,
             dst_rows   [ROWS_TOTAL] original flat output rows, core-major,
             src_rows   [ROWS_TOTAL] staging row per lookup (same order),
             core_of    [ROWS_TOTAL] core id per lookup (same order),
             cnts       [N_CORES, NSUB] int32 per-sub-gather valid counts).
    """
    idx = np.asarray(indices)
    perm = np.asarray(PERM)
    glob = (idx[perm].astype(np.int64) + (perm * V)[:, None]).reshape(-1)

    core = glob // SHARD                       # [N]
    local = glob - core * SHARD
    win = local // WIN                         # window id 0..15
    wlocal = local - win * WIN                 # 0..32767

    group = core * N_WIN + win                 # 0..127
    order = np.lexsort((wlocal, group))        # grouped; row-sorted in group
    g_sorted = group[order]
    w_sorted = wlocal[order]

    # dedup: duplicates of a row within a bucket share one gather slot
    first = np.ones(glob.size, bool)
    first[1:] = (g_sorted[1:] != g_sorted[:-1]) | (w_sorted[1:] != w_sorted[:-1])
    fc = np.cumsum(first) - 1                  # global distinct ordinal
    gstart = np.ones(glob.size, bool)
    gstart[1:] = g_sorted[1:] != g_sorted[:-1]
    gs_idx = np.flatnonzero(gstart)
    run_len = np.diff(np.append(gs_idx, glob.size))
    slot = fc - np.repeat(fc[gs_idx], run_len)  # distinct slot within bucket

    n_groups = N_CORES * N_WIN
    dist_counts = np.zeros(n_groups, np.int64)
    dist_counts[g_sorted[gs_idx]] = (
        fc[np.append(gs_idx[1:] - 1, glob.size - 1)] - fc[gs_idx] + 1
    )
    pads = np.array(PADS * N_CORES)
    if np.any(dist_counts > pads):
        bad = np.argwhere(dist_counts > pads).ravel()
        raise RuntimeError(f"bucket overflow: {bad} {dist_counts[bad]}")

    # staging row: sub-gather s slot r lands at SBUF [p=r%128, col s*8+r//128]
    # of the window tile; the window writeback maps SBUF (p, c) to staging row
    # base_w + p*cols_w + c.
    stage_off = np.cumsum([0] + PADS)[:-1]     # per-window base within a core
    base = np.tile(stage_off, N_CORES)[g_sorted]
    cols = np.array(COLS * N_CORES)[g_sorted]
    sub, r = slot // GRANULE, slot % GRANULE
    src_rows = base + (r % P) * cols + sub * GCOLS + r // P

    # int16 index tiles, wrapped in 16 partitions: distinct slot d goes to
    # [partition d%16, col c0_w + d//16]; unused cells are -1 (skipped by the
    # runtime count register).
    idx_off16 = np.cumsum([0] + [p // 16 for p in PADS])[:-1]
    idx16 = np.full((N_CORES, 16, IDX_COLS), -1, dtype=np.int16)
    c_of = g_sorted // N_WIN
    flat_cols = idx_off16[g_sorted % N_WIN] + slot // 16
    idx16[c_of, slot % 16, flat_cols] = w_sorted.astype(np.int16)

    # per-sub-gather valid counts; empty sub-gathers get one dummy (row 0)
    # because an all-negative gather is illegal.
    sub_of_pad = np.concatenate([[w] * (PADS[w] // GRANULE) for w in range(N_WIN)])
    sub_rank = np.concatenate(
        [np.arange(PADS[w] // GRANULE) for w in range(N_WIN)]
    )
    cnts = np.zeros((N_CORES, NSUB), np.int32)
    for c in range(N_CORES):
        wc = dist_counts[c * N_WIN : (c + 1) * N_WIN]
        sc = np.clip(wc[sub_of_pad] - sub_rank * GRANULE, 0, GRANULE)
        empty = sc == 0
        if np.any(empty):
            gi = np.flatnonzero(empty)
            cells = idx_off16[sub_of_pad[gi]] + sub_rank[gi] * (GRANULE // 16)
            idx16[c, 0, cells] = 0
            sc[gi] = 1
        cnts[c] = sc

    idx_inputs = np.ascontiguousarray(np.tile(idx16, (1, 8, 1)))
    return idx_inputs, order, src_rows, c_of, cnts


_NC_CACHE = {}


def _get_nc():
    if "nc" not in _NC_CACHE:
        _NC_CACHE["nc"] = build_nc()
    return _NC_CACHE["nc"]


def run_sharded(indices, tables, trace=False, **spmd_kwargs):
    """Run the SPMD kernel on 8 cores; returns (full_output, BassKernelResults)."""
    from concourse import bass_utils

    tables_flat = np.asarray(tables, dtype=np.float32).reshape(F * V, D)
    idx_inputs, dst_rows, src_rows, core_of, cnts = route(indices)

    in_maps = []
    for c in range(N_CORES):
        m = {"idx": idx_inputs[c], "cnt": np.tile(cnts[c : c + 1], (P, 1))}
        shard = tables_flat[c * SHARD : (c + 1) * SHARD]
        r0 = 0
        for w in range(N_WIN):
            m[f"tab{w}"] = shard[r0 : r0 + WIN_ROWS[w]]
            r0 += WIN_ROWS[w]
        in_maps.append(m)

    nc = _get_nc()
    res = bass_utils.run_bass_kernel_spmd(
        nc, in_maps, list(range(N_CORES)), trace=trace, **spmd_kwargs
    )

    out_flat = np.empty((ROWS_TOTAL, D), dtype=np.float32)
    for c in range(N_CORES):
        sel = core_of == c
        out_flat[dst_rows[sel]] = res.results[c]["out"][src_rows[sel]].astype(
            np.float32
        )
    return out_flat.reshape(F, BL, D), res


def kernel(indices, tables):
    out, _ = run_sharded(indices, tables, trace=False)
    return out


# revision 14
# speedup vs baseline: 3.5732x; 3.5732x over previous
"""Trainium2 Bass kernel: multi-table embedding gather (pooling=NONE).

Reference computation (hardcoded shapes):
    indices: [F=4, BL=204800] int   (values in [0, V))
    tables:  [F=4, V=1e6, D=64] f32
    out[f]   = tables[PERM[f]][indices[PERM[f]]]   -> [4, 204800, 64] f32
    PERM = [2, 0, 3, 1]

Strategy (model/table-parallel, per the sharding hint):
  * Fold the table permutation into global row ids g = PERM[f]*V + idx over a
    flat [4M, 64] table.
  * Shard the flat table row-wise across the 8 cores (500,000 rows each).
    The host routes every lookup to the core owning its row, bucketing each
    core's lookups by 32,768-row window so the on-core gather can use the
    high-throughput int16 `dma_gather` SWDGE instruction in single-packet
    mode (~0.1us emission per 1024-idx gather; multi-packet mode measured
    ~3ns/desc on the Pool engine and serializes the whole kernel).
  * Each core gathers its (padded) buckets window-by-window into SBUF,
    casts f32 -> fp16 on the otherwise-idle Activation engine, and streams
    the halved bytes to a contiguous fp16 staging output with large HWDGE
    DMAs (writeback HBM traffic 28MB -> 14MB per core; fp16 roundtrip error
    ~5e-4 rel, far inside the 2e-2 gate).
  * The host applies the recorded inverse permutation to scatter staged rows
    into the final [4, 204800, 64] f32 output (host-side unshard + upcast).
"""

import sys

import numpy as np

for _p in ("/opt/trn_rl_repo",):
    if _p not in sys.path:
        sys.path.insert(0, _p)

F = 4
V = 1_000_000
D = 64
BL = 204_800
PERM = (2, 0, 3, 1)

N_CORES = 8
P = 128
ROWS_TOTAL = F * BL                   # 819,200 lookups
SHARD = F * V // N_CORES              # 500,000 table rows per core
WIN = 32_768                          # int16-addressable window
N_FULL_WIN = SHARD // WIN             # 15 full windows
LAST_WIN_ROWS = SHARD - N_FULL_WIN * WIN  # 8,480
N_WIN = N_FULL_WIN + 1                # 16 windows per core

# Per-window bucket capacity (static padding; lookups are uniform so bucket
# sizes concentrate tightly: full-window mean 6711 sigma 82, last-window mean
# 1737 sigma 42).
PAD_FULL = 7_168                      # 56 * 128
PAD_LAST = 2_048                      # 16 * 128
PADS = [PAD_FULL] * N_FULL_WIN + [PAD_LAST]
COLS = [p // P for p in PADS]         # dst free-dim blocks per window
STAGE_ROWS = sum(PADS)                # 109,568 staged rows per core
IDX_COLS = sum(p // 16 for p in PADS)  # int16 idx columns: 6,848
WIN_ROWS = [WIN] * N_FULL_WIN + [LAST_WIN_ROWS]

NBUF = 3                # window dst tiles in flight (21KB/partition each)
GRANULE = 1024          # idxs per dma_gather (single-packet limit: 64 desc/engine)
GCOLS = GRANULE // P    # 8 dst free-dim blocks per sub-gather
N_SWDGE_QUEUES = 4
DMA_SCRATCH = 131072
NSUB = sum(p // GRANULE for p in PADS)   # 107 sub-gathers per core


def build_nc():
    """Per-core SPMD program: dma_gather windows + fp16 cast + writebacks.

    Raw bass (no TileContext): Tile's SWDGE completion tracking rotates 8
    global DMASW lane sems and makes gather i wait for gather i-8's DMA
    completion before issuing, pacing the kernel at ~2.7us/gather (~290us).
    With manual semaphores the gathers issue back-to-back (~0.1us each),
    all 27 per queue fit the 4096-descriptor ring carveout, and the SDMA
    engines stay saturated. Pipeline (NBUF-deep, per window w):
      Pool:   gathers(w) gated on idx-load(w) and writeback(w-NBUF)
      ACT:    cast(w) f32->fp16 gated on gathers(w) and writeback(w-NBUF)
      SP:     writeback(w) gated on cast(w)
    """
    import concourse.bacc as bacc
    import concourse.mybir as mybir

    nc = bacc.Bacc(
        None,
        num_swdge_queues=N_SWDGE_QUEUES,
        dynamic_dma_scratch_size=DMA_SCRATCH,
    )
    tabs = [
        nc.declare_dram_parameter(
            f"tab{w}", [WIN_ROWS[w], D], mybir.dt.float32, isOutput=False
        )
        for w in range(N_WIN)
    ]
    idx_in = nc.declare_dram_parameter(
        "idx", [P, IDX_COLS], mybir.dt.int16, isOutput=False
    )
    cnt_in = nc.declare_dram_parameter(
        "cnt", [P, NSUB], mybir.dt.int32, isOutput=False
    )
    out = nc.declare_dram_parameter(
        "out", [STAGE_ROWS, D], mybir.dt.float16, isOutput=True
    )

    idx_off = np.cumsum([0] + [p // 16 for p in PADS]).tolist()
    stage_off = np.cumsum([0] + PADS).tolist()

    idx_tile = nc.alloc_sbuf_tensor("idx_sb", [P, IDX_COLS], mybir.dt.int16)
    cnt_tile = nc.alloc_sbuf_tensor("cnt_sb", [P, NSUB], mybir.dt.int32)
    datas = [
        nc.alloc_sbuf_tensor(f"data{i}", [P, COLS[0] * D], mybir.dt.float32)
        for i in range(NBUF)
    ]
    halves = [
        nc.alloc_sbuf_tensor(f"half{i}", [P, COLS[0] * D], mybir.dt.float16)
        for i in range(NBUF)
    ]

    regs = [nc.alloc_register(mybir.EngineType.Pool, f"cnt_reg{i}") for i in range(24)]
    csem = nc.alloc_semaphore("csem")   # cnt load,          +16
    isems = [nc.alloc_semaphore(f"isem{w}") for w in range(N_WIN)]
    wsems = [nc.alloc_semaphore(f"wsem{i}") for i in range(NBUF)]
    asem = nc.alloc_semaphore("asem")   # casts,             +1 each
    # completion sems per (window, queue): gathers of window w on queue q
    # inc gsems[w][q] by 16 each and the cast waits for the full per-queue
    # sum, so intermediate values are never waited on and out-of-order
    # completion across gathers is harmless. (An SWDGE sem is locked to a
    # single queue, hence the per-queue split.)
    gsems = [
        [nc.alloc_semaphore(f"gsem{w}_{q}") for q in range(N_SWDGE_QUEUES)]
        for w in range(N_WIN)
    ]

    g_of_w = [PADS[w] // GRANULE for w in range(N_WIN)]
    # per-(window, queue) gather counts for the continuous g_idx % 4 rotation
    n_wq = [[0] * N_SWDGE_QUEUES for _ in range(N_WIN)]
    _g = 0
    for _w in range(N_WIN):
        for _s in range(g_of_w[_w]):
            n_wq[_w][_g % N_SWDGE_QUEUES] += 1
            _g += 1

    with nc.Block("main") as blk:

        @blk.sync
        def _(sync):
            sync.dma_start(out=cnt_tile[:], in_=cnt_in[:]).then_inc(csem, 16)
            # split the idx load per window so window 0 can start gathering
            # without waiting for the full 1.75MB index transfer
            for w in range(N_WIN):
                sync.dma_start(
                    out=idx_tile[:, idx_off[w] : idx_off[w + 1]],
                    in_=idx_in[:, idx_off[w] : idx_off[w + 1]],
                ).then_inc(isems[w], 16)
            for w in range(N_WIN):
                sync.wait_ge(asem, w + 1)
                win_ap = out[stage_off[w] : stage_off[w + 1], :].rearrange(
                    "(p c) d -> p (c d)", p=P
                )
                sync.dma_start(
                    out=win_ap[:], in_=halves[w % NBUF][:, : COLS[w] * D]
                ).then_inc(wsems[w % NBUF], 16)
            for i in range(NBUF):
                sync.wait_ge(wsems[i], 16 * ((N_WIN - 1 - i) // NBUF + 1))

        @blk.gpsimd
        def _(g):
            # Plain (gen0) single-packet gathers: emission is ~0.1us, and the
            # ucode blocks a new gather on a queue until that queue's ring
            # has space. DMA_SCRATCH=131072 sizes each queue ring for ~3
            # single-packet gathers in flight (65 desc x 64B vs 16KB/queue/
            # direction), hiding the ~8.6us posting-to-completion latency
            # that serialized the 65536-scratch variant (1 ring slot/queue).
            g_idx = 0
            g.wait_ge(csem, 16)
            for w in range(N_WIN):
                g.wait_ge(isems[w], 16)
                if w >= NBUF:
                    g.wait_ge(wsems[w % NBUF], 16 * ((w - NBUF) // NBUF + 1))
                for s in range(g_of_w[w]):
                    c0 = idx_off[w] + s * (GRANULE // 16)
                    f0 = s * GRANULE // P * D
                    # Runtime count register: the ucode only emits
                    # descriptors for the valid (deduped) prefix; trailing
                    # -1 idx slots are skipped. 1024 idxs = 64 desc/engine =
                    # the single-packet limit.
                    reg = regs[g_idx % len(regs)]
                    q = g_idx % N_SWDGE_QUEUES
                    g.reg_load(reg, cnt_tile[0:1, g_idx : g_idx + 1])
                    g.dma_gather(
                        datas[w % NBUF][:, f0 : f0 + GCOLS * D].rearrange(
                            "p (c d) -> p c d", d=D
                        ),
                        tabs[w][:],
                        idx_tile[:, c0 : c0 + GRANULE // 16],
                        GRANULE,
                        reg,
                        D,
                        single_packet=True,
                        queue_num=q,
                    ).then_inc(gsems[w][q], 16)
                    g_idx += 1

        @blk.scalar
        def _(sc):
            for w in range(N_WIN):
                for q in range(N_SWDGE_QUEUES):
                    if n_wq[w][q]:
                        sc.wait_ge(gsems[w][q], 16 * n_wq[w][q])
                if w >= NBUF:
                    sc.wait_ge(wsems[w % NBUF], 16 * ((w - NBUF) // NBUF + 1))
                sc.copy(
                    halves[w % NBUF][:, : COLS[w] * D],
                    datas[w % NBUF][:, : COLS[w] * D],
                ).then_inc(asem, 1)

    nc.compile()
    return nc


def route(indices):
    """Host-side routing: global ids -> per-core window buckets.

    Returns (idx_inputs [N_CORES, P, IDX_COLS] int16,
             dst_rows   [ROWS_TOTAL] original flat output rows, core-major,
             src_rows   [ROWS_TOTAL] staging row per lookup (same order),
             core_of    [ROWS_TOTAL] core id per lookup (same order),
             cnts       [N_CORES, NSUB] int32 per-sub-gather valid counts).
    """
    idx = np.asarray(indices)
    perm = np.asarray(PERM)
    glob = (idx[perm].astype(np.int64) + (perm * V)[:, None]).reshape(-1)

    core = glob // SHARD                       # [N]
    local = glob - core * SHARD
    win = local // WIN                         # window id 0..15
    wlocal = local - win * WIN                 # 0..32767

    group = core * N_WIN + win                 # 0..127
    order = np.lexsort((wlocal, group))        # grouped; row-sorted in group
    g_sorted = group[order]
    w_sorted = wlocal[order]

    # dedup: duplicates of a row within a bucket share one gather slot
    first = np.ones(glob.size, bool)
    first[1:] = (g_sorted[1:] != g_sorted[:-1]) | (w_sorted[1:] != w_sorted[:-1])
    fc = np.cumsum(first) - 1                  # global distinct ordinal
    gstart = np.ones(glob.size, bool)
    gstart[1:] = g_sorted[1:] != g_sorted[:-1]
    gs_idx = np.flatnonzero(gstart)
    run_len = np.diff(np.append(gs_idx, glob.size))
    slot = fc - np.repeat(fc[gs_idx], run_len)  # distinct slot within bucket

    n_groups = N_CORES * N_WIN
    dist_counts = np.zeros(n_groups, np.int64)
    dist_counts[g_sorted[gs_idx]] = (
        fc[np.append(gs_idx[1:] - 1, glob.size - 1)] - fc[gs_idx] + 1
    )
    pads = np.array(PADS * N_CORES)
    if np.any(dist_counts > pads):
        bad = np.argwhere(dist_counts > pads).ravel()
        raise RuntimeError(f"bucket overflow: {bad} {dist_counts[bad]}")

    # staging row: sub-gather s slot r lands at SBUF [p=r%128, col s*8+r//128]
    # of the window tile; the window writeback maps SBUF (p, c) to staging row
    # base_w + p*cols_w + c.
    stage_off = np.cumsum([0] + PADS)[:-1]     # per-window base within a core
    base = np.tile(stage_off, N_CORES)[g_sorted]
    cols = np.array(COLS * N_CORES)[g_sorted]
    sub, r = slot // GRANULE, slot % GRANULE
    src_rows = base + (r % P) * cols + sub * GCOLS + r // P

    # int16 index tiles, wrapped in 16 partitions: distinct slot d goes to
    # [partition d%16, col c0_w + d//16]; unused cells are -1 (skipped by the
    # runtime count register).
    idx_off16 = np.cumsum([0] + [p // 16 for p in PADS])[:-1]
    idx16 = np.full((N_CORES, 16, IDX_COLS), -1, dtype=np.int16)
    c_of = g_sorted // N_WIN
    flat_cols = idx_off16[g_sorted % N_WIN] + slot // 16
    idx16[c_of, slot % 16, flat_cols] = w_sorted.astype(np.int16)

    # per-sub-gather valid counts; empty sub-gathers get one dummy (row 0)
    # because an all-negative gather is illegal.
    sub_of_pad = np.concatenate([[w] * (PADS[w] // GRANULE) for w in range(N_WIN)])
    sub_rank = np.concatenate(
        [np.arange(PADS[w] // GRANULE) for w in range(N_WIN)]
    )
    cnts = np.zeros((N_CORES, NSUB), np.int32)
    for c in range(N_CORES):
        wc = dist_counts[c * N_WIN : (c + 1) * N_WIN]
        sc = np.clip(wc[sub_of_pad] - sub_rank * GRANULE, 0, GRANULE)
        empty = sc == 0
        if np.any(empty):
            gi = np.flatnonzero(empty)
            cells = idx_off16[sub_of_pad[gi]] + sub_rank[gi] * (GRANULE // 16)
            idx16[c, 0, cells] = 0
            sc[gi] = 1
        cnts[c] = sc

    idx_inputs = np.ascontiguousarray(np.tile(idx16, (1, 8, 1)))
    return idx_inputs, order, src_rows, c_of, cnts


_NC_CACHE = {}


def _get_nc():
    if "nc" not in _NC_CACHE:
        _NC_CACHE["nc"] = build_nc()
    return _NC_CACHE["nc"]


def run_sharded(indices, tables, trace=False, **spmd_kwargs):
    """Run the SPMD kernel on 8 cores; returns (full_output, BassKernelResults)."""
    from concourse import bass_utils

    tables_flat = np.asarray(tables, dtype=np.float32).reshape(F * V, D)
    idx_inputs, dst_rows, src_rows, core_of, cnts = route(indices)

    in_maps = []
    for c in range(N_CORES):
        m = {"idx": idx_inputs[c], "cnt": np.tile(cnts[c : c + 1], (P, 1))}
        shard = tables_flat[c * SHARD : (c + 1) * SHARD]
        r0 = 0
        for w in range(N_WIN):
            m[f"tab{w}"] = shard[r0 : r0 + WIN_ROWS[w]]
            r0 += WIN_ROWS[w]
        in_maps.append(m)

    nc = _get_nc()
    res = bass_utils.run_bass_kernel_spmd(
        nc, in_maps, list(range(N_CORES)), trace=trace, **spmd_kwargs
    )

    out_flat = np.empty((ROWS_TOTAL, D), dtype=np.float32)
    for c in range(N_CORES):
        sel = core_of == c
        out_flat[dst_rows[sel]] = res.results[c]["out"][src_rows[sel]].astype(
            np.float32
        )
    return out_flat.reshape(F, BL, D), res


def kernel(indices, tables):
    out, _ = run_sharded(indices, tables, trace=False)
    return out
